# revision 3
# baseline (speedup 1.0000x reference)
"""AdaptiveGraphConv Trainium2 kernel: 8-core SPMD, data-parallel over B.

Reference computation (per (b,t) slice over V=25 nodes):
  th = theta(x), ph = phi(x)  (1x1 convs to INTER=32)
  A  = softmax(th @ ph / sqrt(INTER))   (V x V attention)
  out = A @ g(x)                        (g: 1x1 conv to C_OUT=128)
  BatchNorm2d (training stats over (B,T,V)) + affine.

Mapping: each core takes B/8=4 batches. Positions (t,v) are packed 5
t-slices (=125 positions) per PE "group"; scores for the 5 slices are
computed in one 125x125 matmul and block-diagonal-masked after exp.
Z (softmax denom) comes for free from a ones-column appended to g.
Normalize-then-transpose via an identity-rhs matmul gives the (C_OUT,
pos) layout; per-channel sum/sumsq accumulate in PSUM via a ones-lhsT
matmul. BN stats are all-reduced (2*128 floats) across the 8 cores and
applied as a per-channel affine fused into the output stream.

g_b is intentionally dropped: rows of A sum to 1, so +g_b[o] is a
constant per-channel shift that training-mode BN's mean subtraction
cancels exactly.
"""

import sys

sys.path.insert(0, "/opt/trn_rl_repo")

from contextlib import ExitStack

import numpy as np

from concourse import bacc, bass, mybir, tile
from concourse.bass_utils import run_bass_kernel_spmd

B, C_IN, T, V = 32, 64, 300, 25
C_OUT, INTER = 128, 32
EPS = 1e-5
NCORES = 8
BPC = B // NCORES            # batches per core
POS = BPC * T * V            # 30000 positions per core
G = 5                        # t-slices per PE group
GP = G * V                   # 125 positions per group
GW = 4                       # groups fused per wide chunk (500 positions)
WIDE = GW * GP               # 500
NG = POS // GP               # 240 groups per core
NT = B * T * V               # 240000 positions globally (BN denominator)
XCHUNK = 2500                # x stream chunk (cols); 12 chunks per core
OCHUNK = 2500                # output stream chunk; 12 chunks per core
SCALE = 1.0 / float(np.sqrt(INTER))

F32 = mybir.dt.float32
AF = mybir.ActivationFunctionType
ALU = mybir.AluOpType

_CACHE = {}


def _build():
    nc = bacc.Bacc(
        "TRN2", target_bir_lowering=False, debug=False, num_devices=NCORES
    )
    x_d = nc.dram_tensor("x", [C_IN, POS], F32, kind="ExternalInput")
    w2_d = nc.dram_tensor("w2", [C_IN, 2 * INTER], F32, kind="ExternalInput")
    gw_d = nc.dram_tensor("gw", [C_IN, C_OUT], F32, kind="ExternalInput")
    b2_d = nc.dram_tensor("b2", [2 * INTER, 1], F32, kind="ExternalInput")
    mask_d = nc.dram_tensor("mask", [GP, WIDE], F32, kind="ExternalInput")
    eye_d = nc.dram_tensor("eye", [GP, GP], F32, kind="ExternalInput")
    ones_d = nc.dram_tensor("ones", [GP, 1], F32, kind="ExternalInput")
    gb_d = nc.dram_tensor("gamma_beta", [1, 2 * C_OUT], F32, kind="ExternalInput")
    out_d = nc.dram_tensor("out", [C_OUT, POS], F32, kind="ExternalOutput")

    with tile.TileContext(nc) as tc, ExitStack() as ctx:
        const = ctx.enter_context(tc.tile_pool(name="const", bufs=1))
        stash_p = ctx.enter_context(tc.tile_pool(name="stash", bufs=1))
        xp = ctx.enter_context(tc.tile_pool(name="xp", bufs=2))
        wide_p = ctx.enter_context(tc.tile_pool(name="wide", bufs=2))
        work = ctx.enter_context(tc.tile_pool(name="work", bufs=3))
        outp = ctx.enter_context(tc.tile_pool(name="outp", bufs=2))
        ps_proj_p = ctx.enter_context(
            tc.tile_pool(name="psA", bufs=1, space="PSUM")
        )
        ps_s_p = ctx.enter_context(tc.tile_pool(name="psS", bufs=2, space="PSUM"))
        ps_g_p = ctx.enter_context(tc.tile_pool(name="psG", bufs=2, space="PSUM"))
        ps_o_p = ctx.enter_context(tc.tile_pool(name="psO", bufs=1, space="PSUM"))
        ps_y_p = ctx.enter_context(tc.tile_pool(name="psY", bufs=1, space="PSUM"))
        ps_st_p = ctx.enter_context(
            tc.tile_pool(name="psStat", bufs=1, space="PSUM")
        )
        dram = ctx.enter_context(tc.tile_pool(name="dram", bufs=1, space="DRAM"))

        w2 = const.tile([C_IN, 2 * INTER], F32)
        nc.sync.dma_start(w2[:], w2_d[:])
        gw = const.tile([C_IN, C_OUT], F32)
        nc.sync.dma_start(gw[:], gw_d[:])
        b2 = const.tile([2 * INTER, 1], F32)
        nc.sync.dma_start(b2[:], b2_d[:])
        mask = const.tile([GP, WIDE], F32)
        nc.sync.dma_start(mask[:], mask_d[:])
        eye = const.tile([GP, GP], F32)
        nc.sync.dma_start(eye[:], eye_d[:])
        ones = const.tile([GP, 1], F32)
        nc.sync.dma_start(ones[:], ones_d[:])
        gb = const.tile([1, 2 * C_OUT], F32)
        nc.sync.dma_start(gb[:], gb_d[:])

        stash = stash_p.tile([C_OUT, POS], F32)
        ps_stats = ps_st_p.tile([1, 2 * C_OUT], F32)

        gabs = 0
        for ci in range(POS // XCHUNK):
            x_sb = xp.tile([C_IN, XCHUNK], F32)
            nc.sync.dma_start(x_sb[:], x_d[:, ci * XCHUNK : (ci + 1) * XCHUNK])
            for wj in range(XCHUNK // WIDE):
                xoff = wj * WIDE
                ps_proj = ps_proj_p.tile([2 * INTER, WIDE], F32)
                nc.tensor.matmul(
                    ps_proj[:], w2[:], x_sb[:, xoff : xoff + WIDE],
                    start=True, stop=True,
                )
                th = wide_p.tile([INTER, WIDE], F32, tag="th")
                ph = wide_p.tile([INTER, WIDE], F32, tag="ph")
                nc.scalar.activation(
                    th[:], ps_proj[0:INTER, :], AF.Identity, bias=b2[0:INTER, :]
                )
                nc.scalar.activation(
                    ph[:], ps_proj[INTER : 2 * INTER, :], AF.Identity,
                    bias=b2[INTER : 2 * INTER, :],
                )
                ps_s = ps_s_p.tile([GP, WIDE], F32)
                for j in range(GW):
                    sl = slice(j * GP, (j + 1) * GP)
                    # scoresT[w, v] = sum_i ph[i, w] * th[i, v]
                    nc.tensor.matmul(
                        ps_s[:, sl], ph[:, sl], th[:, sl], start=True, stop=True
                    )
                pexp = wide_p.tile([GP, WIDE], F32, tag="pexp")
                nc.scalar.activation(pexp[:], ps_s[:], AF.Exp, scale=SCALE)
                pmT = wide_p.tile([GP, WIDE], F32, tag="pmT")
                nc.vector.tensor_mul(pmT[:], pexp[:], mask[:])
                for j in range(GW):
                    pos0 = ci * XCHUNK + xoff + j * GP
                    ps_g = ps_g_p.tile([GP, C_OUT], F32)
                    nc.tensor.matmul(
                        ps_g[:],
                        x_sb[:, xoff + j * GP : xoff + (j + 1) * GP],
                        gw[:],
                        start=True, stop=True,
                    )
                    g_sb = work.tile([GP, C_OUT + 1], F32, tag="g_sb")
                    nc.scalar.activation(g_sb[:, 0:C_OUT], ps_g[:], AF.Copy)
                    nc.gpsimd.memset(g_sb[:, C_OUT : C_OUT + 1], 1.0)
                    ps_o = ps_o_p.tile([GP, C_OUT + 1], F32)
                    nc.tensor.matmul(
                        ps_o[:], pmT[:, j * GP : (j + 1) * GP], g_sb[:],
                        start=True, stop=True,
                    )
                    rz = work.tile([GP, 1], F32, tag="rz")
                    nc.vector.reciprocal(rz[:], ps_o[:, C_OUT : C_OUT + 1])
                    stat_in = work.tile([GP, 2 * C_OUT], F32, tag="stat_in")
                    nc.vector.tensor_scalar_mul(
                        stat_in[:, 0:C_OUT], ps_o[:, 0:C_OUT], rz[:]
                    )
                    nc.scalar.square(
                        stat_in[:, C_OUT : 2 * C_OUT], stat_in[:, 0:C_OUT]
                    )
                    nc.tensor.matmul(
                        ps_stats[:], ones[:], stat_in[:],
                        start=(gabs == 0), stop=(gabs == NG - 1),
                    )
                    ps_y = ps_y_p.tile([C_OUT, GP], F32, tag="ps_y")
                    nc.tensor.matmul(
                        ps_y[:], stat_in[:, 0:C_OUT], eye[:], start=True, stop=True
                    )
                    nc.vector.tensor_copy(stash[:, pos0 : pos0 + GP], ps_y[:])
                    gabs += 1

        # ---- phase 2: BN stats all-reduce + per-channel affine coefs ----
        stats_sb = work.tile([1, 2 * C_OUT], F32, tag="stats_sb")
        nc.vector.tensor_copy(stats_sb[:], ps_stats[:])
        cc_in = dram.tile([1, 2 * C_OUT], F32)
        cc_out = dram.tile([1, 2 * C_OUT], F32)
        nc.sync.dma_start(cc_in[:], stats_sb[:])
        nc.gpsimd.collective_compute(
            "AllReduce",
            ALU.add,
            replica_groups=[list(range(NCORES))],
            ins=[cc_in.opt()],
            outs=[cc_out.opt()],
        )
        gstats = work.tile([1, 2 * C_OUT], F32, tag="gstats")
        nc.sync.dma_start(gstats[:], cc_out[:])
        # mean row, E[y^2] row
        mrow = work.tile([1, C_OUT], F32, tag="mrow")
        nc.vector.tensor_scalar_mul(mrow[:], gstats[:, 0:C_OUT], 1.0 / NT)
        vrow = work.tile([1, C_OUT], F32, tag="vrow")
        nc.vector.tensor_scalar_mul(vrow[:], gstats[:, C_OUT:], 1.0 / NT)
        m2row = work.tile([1, C_OUT], F32, tag="m2row")
        nc.scalar.square(m2row[:], mrow[:])
        nc.vector.tensor_sub(vrow[:], vrow[:], m2row[:])  # var = E[y^2]-mean^2
        nc.vector.tensor_scalar_add(vrow[:], vrow[:], float(EPS))
        srow = work.tile([1, C_OUT], F32, tag="srow")
        nc.scalar.activation(srow[:], vrow[:], AF.Sqrt)
        nc.vector.reciprocal(srow[:], srow[:])            # rstd
        nc.vector.tensor_mul(srow[:], srow[:], gb[:, 0:C_OUT])  # s = gamma*rstd
        crow = work.tile([1, C_OUT], F32, tag="crow")
        nc.vector.tensor_mul(crow[:], mrow[:], srow[:])
        nc.vector.tensor_sub(crow[:], gb[:, C_OUT:], crow[:])  # c = beta - mean*s
        # transpose (1,128) rows -> (128,1) cols via K=1 matmuls
        ps_sc = ps_y_p.tile([C_OUT, 2], F32, tag="ps_y")
        nc.tensor.matmul(ps_sc[:, 0:1], srow[:], ones[0:1, :], start=True, stop=True)
        nc.tensor.matmul(ps_sc[:, 1:2], crow[:], ones[0:1, :], start=True, stop=True)
        scol = work.tile([C_OUT, 1], F32, tag="scol")
        ccol = work.tile([C_OUT, 1], F32, tag="ccol")
        nc.vector.tensor_copy(scol[:], ps_sc[:, 0:1])
        nc.vector.tensor_copy(ccol[:], ps_sc[:, 1:2])

        # ---- phase 3: BN apply fused into output stream ----
        for ck in range(POS // OCHUNK):
            ob = outp.tile([C_OUT, OCHUNK], F32)
            nc.vector.tensor_scalar(
                ob[:],
                stash[:, ck * OCHUNK : (ck + 1) * OCHUNK],
                scol[:],
                ccol[:],
                ALU.mult,
                ALU.add,
            )
            nc.sync.dma_start(out_d[:, ck * OCHUNK : (ck + 1) * OCHUNK], ob[:])

    nc.compile()
    return nc


def _consts():
    mask = np.zeros((GP, WIDE), dtype=np.float32)
    for j in range(GW):
        for p in range(GP):
            s = p // V
            mask[p, j * GP + s * V : j * GP + (s + 1) * V] = 1.0
    # mask[p, j*GP+q] = 1 iff p//V == q//V; built above row-wise:
    # row p belongs to slice s=p//V -> cols of slice s in each group j.
    # But that sets mask[p, cols of slice s] which is exactly p//V==q//V. OK.
    eye = np.eye(GP, dtype=np.float32)
    ones = np.ones((GP, 1), dtype=np.float32)
    return mask, eye, ones


def kernel(x, theta_w, theta_b, phi_w, phi_b, g_w, g_b, bn_gamma, bn_beta):
    x = np.asarray(x, dtype=np.float32)
    if "nc" not in _CACHE:
        _CACHE["nc"] = _build()
    nc = _CACHE["nc"]

    w2 = np.concatenate(
        [np.asarray(theta_w).T, np.asarray(phi_w).T], axis=1
    ).astype(np.float32)  # (C_IN, 64)
    gwm = np.asarray(g_w).T.astype(np.float32).copy()  # (C_IN, C_OUT)
    b2 = np.concatenate([np.asarray(theta_b), np.asarray(phi_b)])[
        :, None
    ].astype(np.float32)
    mask, eye, ones = _consts()
    gb = np.concatenate([np.asarray(bn_gamma), np.asarray(bn_beta)])[
        None, :
    ].astype(np.float32)

    in_maps = []
    for c in range(NCORES):
        xs = (
            x[c * BPC : (c + 1) * BPC]
            .transpose(1, 0, 2, 3)
            .reshape(C_IN, POS)
            .copy()
        )
        in_maps.append(
            {
                "x": xs,
                "w2": w2,
                "gw": gwm,
                "b2": b2,
                "mask": mask,
                "eye": eye,
                "ones": ones,
                "gamma_beta": gb,
            }
        )

    res = run_bass_kernel_spmd(nc, in_maps, core_ids=list(range(NCORES)))
    out = np.empty((B, C_OUT, T, V), dtype=np.float32)
    for c in range(NCORES):
        oc = res.results[c]["out"]  # (C_OUT, POS), b-major positions
        out[c * BPC : (c + 1) * BPC] = (
            oc.reshape(C_OUT, BPC, T, V).transpose(1, 0, 2, 3)
        )
    return out
